# revision 49
# baseline (speedup 1.0000x reference)
"""CTC prefix scorer on Trainium2 — Bass/Tile kernel, SPMD over 8 NeuronCores.

Math: the reference's 490-step lax.scan result is dead code (its output `r`
is only read at row start-1, which always comes from the LOGZERO prefix /
t=0 init), so the whole computation collapses to, per hypothesis h:

  log_psi[h, c] = log( sum_t w[t, h] * exp(x[b_h, t, c]) )

where w[t, h] = exp(rsum[t-1, h]) * [start <= t < xlen_b]  (normal labels)
          or  = exp(r_prev[t-1, 1, h]) * [...]             (c == last_ids[h])
with rsum = logaddexp(r_prev[:,0], r_prev[:,1]).  That is a (16 x T) @
(T x O) matmul per batch.  Frame masking folds into w (masked frames only
affect the BLANK/EOS output columns, which are overwritten anyway).  Final
output: scatter-select scored columns, EOS column = rsum[xlen-1], BLANK
column = LOGZERO, minus s_prev.

Sharding: core i <-> batch i (its 8 hypotheses).  x fully sharded on B.
"""

import numpy as np
from contextlib import ExitStack

import concourse.bass as bass
import concourse.tile as tile
from concourse import bacc, mybir
from concourse.bass import IndirectOffsetOnAxis
from concourse.bass_utils import run_bass_kernel_spmd
from concourse.tile_rust import add_dep_helper as _add_dep


def add_dep_helper(a, b, reason=""):
    """a depends on b; unwrap BassInstruction -> mybir.Instruction."""
    _add_dep(getattr(a, "ins", a), getattr(b, "ins", b), reason=reason)

F32 = mybir.dt.float32
F32R = mybir.dt.float32r
I32 = mybir.dt.int32
ACT = mybir.ActivationFunctionType
ALU = mybir.AluOpType

B, T, O = 8, 500, 10000
NH = 8                       # hypotheses per batch == per core
NCORES = 8
LOGZERO = -1e10
BLANK, EOS = 0, 2
SNUM = 200

NT = 512                     # N-tile width (one PSUM bank of f32)
N_TILES = [(c0, min(NT, O - c0)) for c0 in range(0, O, NT)]
K_CHUNKS = [(t0, min(128, T - t0)) for t0 in range(0, T, 128)]  # K over t<=499
NSCAT = (NH * SNUM + 127) // 128          # 13 indirect-scatter calls


def build_nc(start: int) -> bass.Bass:
    import os
    PHASE = os.environ.get("KDEBUG_PHASE", "full")
    nc = bacc.Bacc(None)
    x_d = nc.declare_dram_parameter("x", [T, O], F32, isOutput=False)
    rp_d = nc.declare_dram_parameter("rprev", [T, 2 * NH], F32, isOutput=False)
    sp_d = nc.declare_dram_parameter("sprev", [NH, O], F32, isOutput=False)
    li_d = nc.declare_dram_parameter("lastids", [NH, 1], I32, isOutput=False)
    mask_d = nc.declare_dram_parameter("smask", [NH, O], I32, isOutput=False)
    xl_d = nc.declare_dram_parameter("xlen", [128, 1], I32, isOutput=False)
    out_d = nc.declare_dram_parameter("out", [NH, O], F32, isOutput=True)

    with ExitStack() as ctx:
        tc = ctx.enter_context(tile.TileContext(nc))
        persist = ctx.enter_context(tc.tile_pool(name="persist", bufs=1))
        xpool = ctx.enter_context(tc.tile_pool(name="xp", bufs=8))
        psum = ctx.enter_context(tc.tile_pool(name="ps", bufs=4, space="PSUM"))
        epi = ctx.enter_context(tc.tile_pool(name="epi", bufs=6))

        # ---------------- xlen broadcast ------------------------------------
        xlb = persist.tile([128, 1], I32, tag="xlb")
        nc.sync.dma_start(out=xlb[:], in_=xl_d[:, :])
        xlb_f = persist.tile([128, 1], F32, tag="xlbf")
        nc.vector.tensor_copy(out=xlb_f[:], in_=xlb[:])

        # ---------------- eos score (DVE/ACT only, no PE) -------------------
        # eos[h] = rsum[xlen-1, h] = log(exp(r0[e,h]) + exp(r1[e,h]));
        # select row e = xlen-1 via a one-hot multiply + free-dim reduce.
        rpt = persist.tile([NH, 2 * T], F32, tag="rpt")
        nc.sync.dma_start(out=rpt[:, 0:T],
                          in_=rp_d[:, 0:NH].rearrange("t h -> h t"))
        nc.sync.dma_start(out=rpt[:, T:2 * T],
                          in_=rp_d[:, NH:2 * NH].rearrange("t h -> h t"))
        ept = persist.tile([NH, 2 * T], F32, tag="ept")
        nc.scalar.activation(ept[:], rpt[:], ACT.Exp)
        esum = persist.tile([NH, T], F32, tag="esum")
        nc.vector.tensor_tensor(out=esum[:], in0=ept[:, 0:T],
                                in1=ept[:, T:2 * T], op=ALU.add)
        iot_i = persist.tile([NH, T], I32, tag="ioti")
        nc.gpsimd.iota(iot_i[:], pattern=[[1, T]], base=0, channel_multiplier=0)
        iot_f = persist.tile([NH, T], F32, tag="iotf")
        nc.vector.tensor_copy(out=iot_f[:], in_=iot_i[:])
        ohm = persist.tile([NH, T], F32, tag="ohm")
        nc.vector.tensor_scalar(out=ohm[:], in0=iot_f[:], scalar1=1.0,
                                scalar2=xlb_f[0:NH, :1], op0=ALU.add,
                                op1=ALU.is_equal)  # (t+1)==xlen
        emsk = persist.tile([NH, T], F32, tag="emsk")
        nc.vector.tensor_tensor(out=emsk[:], in0=esum[:], in1=ohm[:],
                                op=ALU.mult)
        esel = persist.tile([NH, 1], F32, tag="esel")
        nc.vector.reduce_sum(out=esel[:], in_=emsk[:],
                             axis=mybir.AxisListType.X)
        eos_sb = persist.tile([NH, 1], F32, tag="eos")
        nc.scalar.activation(eos_sb[:], esel[:], ACT.Ln)

        # ---------------- lhsT weights --------------------------------------
        # lhsT row t (global) <- r_prev[t-1]; chunk k covers t in [128k,128k+128)
        lhsTs = []
        for k, (t0, _) in enumerate(K_CHUNKS):
            a, b = max(t0, 1), min(t0 + 128, T)
            pa, pb = a - t0, b - t0
            # full-128-partition ops only (SBUF compute APs must start at
            # partition 0): unloaded rows hold exp(0)=1 etc., neutralized by
            # the wm/oh masks below (always 0 there).
            e_t = persist.tile([128, 2 * NH], F32, tag=f"e{k}")
            nc.gpsimd.memset(e_t[:], 0.0)
            nc.sync.dma_start(out=e_t[pa:pb, :], in_=rp_d[a - 1:b - 1, :])
            nc.scalar.activation(e_t[:], e_t[:], ACT.Exp)
            sum_t = persist.tile([128, NH], F32, tag=f"sum{k}")
            nc.vector.tensor_tensor(out=sum_t[:], in0=e_t[:, 0:NH],
                                    in1=e_t[:, NH:2 * NH], op=ALU.add)

            io_t = persist.tile([128, 1], I32, tag=f"io{k}")
            nc.gpsimd.iota(io_t[:], pattern=[[0, 1]], base=t0, channel_multiplier=1)
            io_f = persist.tile([128, 1], F32, tag=f"iof{k}")
            nc.vector.tensor_copy(out=io_f[:], in_=io_t[:])
            ge_t = persist.tile([128, 1], F32, tag=f"ge{k}")
            nc.vector.tensor_scalar(out=ge_t[:], in0=io_f[:], scalar1=float(start),
                                    scalar2=None, op0=ALU.is_ge)
            lt_t = persist.tile([128, 1], F32, tag=f"lt{k}")
            nc.vector.tensor_scalar(out=lt_t[:], in0=io_f[:], scalar1=xlb_f[:, :1],
                                    scalar2=None, op0=ALU.is_lt)
            wm_t = persist.tile([128, 1], F32, tag=f"wm{k}")
            nc.vector.tensor_tensor(out=wm_t[:], in0=ge_t[:], in1=lt_t[:],
                                    op=ALU.mult)

            # w1 half lives at col 32 so the matmul output lands at PSUM
            # partition 32 (hardware requires partition starts in {0,32,64,96})
            w_t = persist.tile([128, 32 + NH], F32R, tag=f"w{k}")
            nc.vector.tensor_scalar(out=w_t[:, NH:32],
                                    in0=wm_t[:, :1].to_broadcast([128, 32 - NH]),
                                    scalar1=0.0, scalar2=None, op0=ALU.mult)
            nc.vector.tensor_scalar(out=w_t[:, 0:NH], in0=sum_t[:], scalar1=wm_t[:, :1],
                                    scalar2=None, op0=ALU.mult)
            nc.vector.tensor_scalar(out=w_t[:, 32:32 + NH], in0=e_t[:, NH:2 * NH],
                                    scalar1=wm_t[:, :1], scalar2=None, op0=ALU.mult)
            lhsTs.append(w_t)

        # ---------------- shared epilogue constants -------------------------
        iotac_i = persist.tile([NH, NT], I32, tag="iotaci")
        nc.gpsimd.iota(iotac_i[:], pattern=[[1, NT]], base=0, channel_multiplier=0)
        iotac = persist.tile([NH, NT], F32, tag="iotac")
        nc.vector.tensor_copy(out=iotac[:], in_=iotac_i[:])
        lz_t = persist.tile([NH, NT], F32, tag="lz")
        nc.gpsimd.memset(lz_t[:], LOGZERO)
        li_t = persist.tile([NH, 1], I32, tag="li")
        nc.sync.dma_start(out=li_t[:], in_=li_d[:, :])
        li_f = persist.tile([NH, 1], F32, tag="lif")
        nc.vector.tensor_copy(out=li_f[:], in_=li_t[:])

        # ---------------- main loop over N-tiles ----------------------------
        for j, (c0, N) in enumerate(N_TILES):
            acc = psum.tile([32 + NH, NT], F32, tag="acc")
            for k, (t0, K) in enumerate(K_CHUNKS):
                xraw = xpool.tile([128, NT], F32, tag="xraw")
                nc.sync.dma_start(out=xraw[:K, :N],
                                  in_=x_d[t0:t0 + K, c0:c0 + N])
                xt = xpool.tile([128, NT], F32R, tag="xt")
                nc.scalar.activation(xt[:K, :N], xraw[:K, :N], ACT.Exp)
                nc.tensor.matmul(out=acc[:, :N], lhsT=lhsTs[k][:K, :],
                                 rhs=xt[:K, :N],
                                 start=(k == 0), stop=(k == len(K_CHUNKS) - 1))

            la0 = epi.tile([NH, NT], F32, tag="la0")
            la1 = epi.tile([NH, NT], F32, tag="la1")
            nc.scalar.activation(la0[:, :N], acc[0:NH, :N], ACT.Ln)
            nc.scalar.activation(la1[:, :N], acc[32:32 + NH, :N], ACT.Ln)
            if PHASE == "mmonly":
                nc.sync.dma_start(out=out_d[:, c0:c0 + N], in_=la0[:, :N])
                continue

            lastc0 = epi.tile([NH, 1], F32, tag="lastc0")
            nc.vector.tensor_scalar(out=lastc0[:], in0=li_f[:], scalar1=float(c0),
                                    scalar2=None, op0=ALU.subtract)
            hit = epi.tile([NH, NT], I32, tag="hit")
            nc.vector.tensor_scalar(out=hit[:, :N], in0=iotac[:, :N],
                                    scalar1=lastc0[:, :1], scalar2=None,
                                    op0=ALU.is_equal)
            val = epi.tile([NH, NT], F32, tag="val")
            nc.vector.tensor_copy(out=val[:, :N], in_=la0[:, :N])
            nc.vector.copy_predicated(out=val[:, :N], mask=hit[:, :N],
                                      data=la1[:, :N])

            sm = epi.tile([NH, NT], I32, tag="sm")
            nc.sync.dma_start(out=sm[:, :N], in_=mask_d[:, c0:c0 + N])
            spv = epi.tile([NH, NT], F32, tag="spv")
            nc.sync.dma_start(out=spv[:, :N], in_=sp_d[:, c0:c0 + N])

            fin = epi.tile([NH, NT], F32, tag="fin")
            nc.vector.tensor_copy(out=fin[:, :N], in_=lz_t[:, :N])
            nc.vector.copy_predicated(out=fin[:, :N], mask=sm[:, :N],
                                      data=val[:, :N])
            nc.vector.tensor_tensor(out=fin[:, :N], in0=fin[:, :N],
                                    in1=spv[:, :N], op=ALU.subtract)
            if j == 0:
                # BLANK col: LOGZERO - s_prev;  EOS col: eos - s_prev
                nc.vector.tensor_scalar(out=fin[:, BLANK:BLANK + 1],
                                        in0=spv[:, BLANK:BLANK + 1],
                                        scalar1=-1.0, scalar2=LOGZERO,
                                        op0=ALU.mult, op1=ALU.add)
                nc.vector.tensor_tensor(out=fin[:, EOS:EOS + 1], in0=eos_sb[:],
                                        in1=spv[:, EOS:EOS + 1], op=ALU.subtract)
            nc.sync.dma_start(out=out_d[:, c0:c0 + N], in_=fin[:, :N])

    nc.compile()
    return nc


def make_in_maps(x, r_prev, s_prev, xlens, last_ids, scoring_ids):
    """Per-core input maps: core i owns batch i / hypotheses [8i, 8i+8)."""
    in_maps = []
    for i in range(NCORES):
        hs = slice(i * NH, (i + 1) * NH)
        sids = np.ascontiguousarray(scoring_ids[hs]).astype(np.int64)  # (8,200)
        smask = np.zeros((NH, O), np.int32)
        np.put_along_axis(smask, sids, 1, axis=1)
        in_maps.append({
            "x": np.ascontiguousarray(x[i]).astype(np.float32),
            "rprev": np.ascontiguousarray(r_prev[:, :, hs]).reshape(T, 2 * NH).astype(np.float32),
            "sprev": np.ascontiguousarray(s_prev[hs]).astype(np.float32),
            "lastids": np.ascontiguousarray(last_ids[hs]).astype(np.int32)[:, None],
            "smask": smask,
            "xlen": np.full((128, 1), int(xlens[i]), np.int32),
        })
    return in_maps


_NC_CACHE: dict[int, bass.Bass] = {}


def kernel(x, r_prev, s_prev, xlens, last_ids, scoring_ids, output_length,
           _trace=False):
    x = np.asarray(x)
    r_prev = np.asarray(r_prev)
    s_prev = np.asarray(s_prev)
    xlens = np.asarray(xlens)
    last_ids = np.asarray(last_ids)
    scoring_ids = np.asarray(scoring_ids)
    start = max(int(output_length), 1)
    # output_length == 0 adds an extra x_[0,0] term; inputs here always have
    # output_length >= 1, which this kernel implements.
    assert int(output_length) >= 1, "output_length==0 path not implemented"

    if start not in _NC_CACHE:
        _NC_CACHE[start] = build_nc(start)
    nc = _NC_CACHE[start]

    in_maps = make_in_maps(x, r_prev, s_prev, xlens, last_ids, scoring_ids)
    res = run_bass_kernel_spmd(nc, in_maps, core_ids=list(range(NCORES)),
                               trace=_trace)
    out = np.concatenate([res.results[i]["out"] for i in range(NCORES)], axis=0)
    kernel.last_exec_time_ns = res.exec_time_ns
    kernel.last_results = res
    return out.astype(np.float32)
